# revision 21
# baseline (speedup 1.0000x reference)
"""Trainium2 Bass kernel for flax MultiHeadDotProductAttention.

Shapes (hardcoded): B=4, Q=K=1500, D=1024, H=16, HD=64.
Sharding: 8 cores = 4 batches x 2 head-groups (8 heads each).
Each core computes its batch's attention output for its 8 heads plus the
output projection restricted to those heads; the host sums the two
head-group partials per batch and adds bo.

Schedule (v2):
  - x DMAs are column-split per q-chunk so the first k-projection group
    starts ~1us in and attention pair 0 starts as soon as kT block 0 /
    qT block 0 cols 0-511 exist (~20us, vs ~49us when phase 1 is a
    monolithic barrier). This pulls the exp stream forward: the Scalar
    engine (softmax exp, ~160us busy) is the co-bottleneck, so it must
    start early and never starve.
  - Remaining projection groups (kT blocks 1-3, qT all blocks/chunks, v
    per k-tile) and the PREVIOUS chunk's output projection are emitted
    as fillers pinned to specific (chunk, pair, kt) attention steps:
    each step is exp-paced (~1.05us) but only has ~0.66us of attention
    matmuls, so fillers absorb the slack instead of idling the PE.
  - Within a step: S^T row-tiled pair first (so exp issues immediately),
    then fillers, then the attn@V pair for the previous k-tile.
  - Softmax denominator rides the attn@V stationary as ones-columns at
    M=64 and M=96; a DVE stream_shuffle broadcasts psum rows 64/96 to
    64 partitions (mask [0]*32 covers both 32-quadrants), replacing the
    f32r selector matmuls of v1 (which cost ~430ns each on the PE and
    serialized its weight load).
  - PSUM budget: st pool 2 tiles x 2 banks (S^T double-buffer; proj/v/
    outproj groups borrow these slots between steps) + attn accumulator
    pool 4 x 1 bank (pe/po per pair, two pairs overlap) = 8 banks.
"""

import os
import sys

sys.path.insert(0, "/opt/trn_rl_repo")

import numpy as np  # noqa: E402
import ml_dtypes  # noqa: E402
import concourse.bacc as bacc  # noqa: E402
import concourse.mybir as mybir  # noqa: E402
import concourse.tile as tile  # noqa: E402
from concourse.bass_utils import run_bass_kernel_spmd  # noqa: E402

F32 = mybir.dt.float32
BF16 = mybir.dt.bfloat16
AF = mybir.ActivationFunctionType

B, SEQ, D, H, HD = 4, 1500, 1024, 16, 64
HG = 8                      # heads per group
HHD = HG * HD               # 512
DCH = D // 128              # 8 d-chunks
HB = HHD // 128             # 4 hhd blocks (2 heads each)
NPAIR = HB                  # 4 head pairs per group
QC = [(0, 512), (512, 512), (1024, 476)]          # q chunks
NQC = len(QC)
KT = [(i * 128, min(128, SEQ - i * 128)) for i in range((SEQ + 127) // 128)]
NKT = len(KT)               # 12 (last tile 92 rows)
VW = 65                     # v stationary width: 64 hd + den ones-col at 64


def _build(with_bias):
    nc = bacc.Bacc("TRN2", target_bir_lowering=False, debug=False, num_devices=8)

    xqT = nc.declare_dram_parameter("xqT", [D, SEQ], BF16, isOutput=False)
    xkvT = nc.declare_dram_parameter("xkvT", [D, SEQ], BF16, isOutput=False)
    wq_d = nc.declare_dram_parameter("wq", [D, HHD], BF16, isOutput=False)
    wk_d = nc.declare_dram_parameter("wk", [D, HHD], BF16, isOutput=False)
    wv_d = nc.declare_dram_parameter("wv", [D, HHD], BF16, isOutput=False)
    wo_d = nc.declare_dram_parameter("wo", [HHD, D], BF16, isOutput=False)
    bq_d = nc.declare_dram_parameter("bq", [1, HHD], BF16, isOutput=False)
    bk_d = nc.declare_dram_parameter("bk", [1, HHD], BF16, isOutput=False)
    bv_d = nc.declare_dram_parameter("bv", [1, HHD], BF16, isOutput=False)
    out_d = nc.declare_dram_parameter("out", [SEQ, D], F32, isOutput=True)

    with tile.TileContext(nc) as tc:
        from contextlib import ExitStack

        with ExitStack() as ctx:
            ctx.enter_context(nc.allow_low_precision(
                reason="bf16 matmul operands; psum accumulation is fp32"
            ))
            const = ctx.enter_context(tc.tile_pool(name="const", bufs=1))

            # ---------------- DMA emission (ordered by first use) ------------
            wk_d3 = wk_d.rearrange("(c p) n -> c p n", p=128)
            wq_d3 = wq_d.rearrange("(c p) n -> c p n", p=128)
            wv_d3 = wv_d.rearrange("(c p) n -> c p n", p=128)

            # wk/wq split at head-block 0: the first S^T only needs block-0
            # columns, so 2.6MB (not 4.2MB) gates the first exp
            wkA = const.tile([128, DCH, 128], BF16, tag="wkA")
            nc.sync.dma_start(
                wkA[:], wk_d[:, 0:128].rearrange("(c p) n -> p c n", p=128))
            wkB = const.tile([128, DCH, HHD - 128], BF16, tag="wkB")
            if with_bias:
                ones_r = const.tile([1, 512], BF16, tag="ones")
                nc.vector.memset(ones_r[:], 1.0)
                bq_sb = const.tile([1, HHD], BF16, tag="bq")
                bk_sb = const.tile([1, HHD], BF16, tag="bk")
                bv_sb = const.tile([1, HHD], BF16, tag="bv")
                nc.sync.dma_start(bq_sb[:], bq_d[:])
                nc.sync.dma_start(bk_sb[:], bk_d[:])
                nc.sync.dma_start(bv_sb[:], bv_d[:])
            else:
                ones_r = bq_sb = bk_sb = bv_sb = None

            # x loads: one DMA for cols 0:512 of all 8 c-chunks (the
            # minimal prefix for an early attention start) and one for cols
            # 512:1500; single issues keep the sync engine's ~0.7us per-DMA
            # issue cost off the critical path. xs[qci][c] resolves to the
            # [128, cw] view for that q-chunk / c-chunk.
            xkv_p = [[None] * DCH for _ in range(NQC)]
            xq_p = [[None] * DCH for _ in range(NQC)]

            def load_x_region(dst, dram, qci):
                lo, w = QC[qci]
                t = const.tile([128, DCH, w], BF16, tag=f"{dst}{qci}",
                               name=f"{dst}_{qci}")
                nc.sync.dma_start(
                    t[:],
                    dram[:, lo:lo + w].rearrange("(c p) n -> p c n", p=128),
                )
                return t

            xkv0_t = const.tile([128, DCH, 512], BF16, tag="xkv0")
            for c in range(DCH):
                nc.sync.dma_start(xkv0_t[:, c, :],
                                  xkvT[c * 128:(c + 1) * 128, 0:512])
                xkv_p[0][c] = xkv0_t[:, c, :]
            wqA = const.tile([128, DCH, 128], BF16, tag="wqA")
            nc.sync.dma_start(
                wqA[:], wq_d[:, 0:128].rearrange("(c p) n -> p c n", p=128))
            wqB = const.tile([128, DCH, HHD - 128], BF16, tag="wqB")
            xq0_t = const.tile([128, DCH, 512], BF16, tag="xq0")
            for c in range(DCH):
                nc.sync.dma_start(xq0_t[:, c, :],
                                  xqT[c * 128:(c + 1) * 128, 0:512])
                xq_p[0][c] = xq0_t[:, c, :]
            wv_t = const.tile([128, DCH, HHD], BF16, tag="wv")
            nc.sync.dma_start(wv_t[:], wv_d.rearrange("(c p) n -> p c n", p=128))
            wv_sb = [wv_t[:, c, :] for c in range(DCH)]
            xkv1_t = load_x_region("xkv", xkvT, 1)
            for c in range(DCH):
                xkv_p[1][c] = xkv1_t[:, c, :]
            nc.sync.dma_start(
                wkB[:], wk_d[:, 128:HHD].rearrange("(c p) n -> p c n", p=128))
            nc.sync.dma_start(
                wqB[:], wq_d[:, 128:HHD].rearrange("(c p) n -> p c n", p=128))
            xkv2_t = load_x_region("xkv", xkvT, 2)
            for c in range(DCH):
                xkv_p[2][c] = xkv2_t[:, c, :]
            xq1_t = load_x_region("xq", xqT, 1)
            xq2_t = load_x_region("xq", xqT, 2)
            for c in range(DCH):
                xq_p[1][c] = xq1_t[:, c, :]
                xq_p[2][c] = xq2_t[:, c, :]
            wo_sb = const.tile([128, HB, D], BF16, tag="wo")
            nc.sync.dma_start(
                wo_sb[:], wo_d.rearrange("(c p) n -> p c n", p=128)
            )

            # ---------------- persistent activation tiles --------------------
            qT_b = [const.tile([128, SEQ], BF16, tag=f"qT{i}", name=f"qT{i}")
                    for i in range(HB)]
            kT = const.tile([128, HB, SEQ], BF16, tag="kT")
            v_t = []
            for kt in range(NKT):
                vt = const.tile([128, HG, VW], BF16, tag=f"v{kt}",
                                name=f"v{kt}")
                nc.vector.memset(vt[:, :, 64:65], 1.0)
                v_t.append(vt)

            # ---------------- pools -----------------------------------------
            st_ps = ctx.enter_context(
                tc.tile_pool(name="stps", bufs=2, space="PSUM"))
            at_ps = ctx.enter_context(
                tc.tile_pool(name="atps", bufs=3, space="PSUM"))
            fill_ps = ctx.enter_context(
                tc.tile_pool(name="fillps", bufs=1, space="PSUM"))
            p_pool = ctx.enter_context(tc.tile_pool(name="p", bufs=8))
            an_pool = ctx.enter_context(tc.tile_pool(name="an", bufs=8))
            small = ctx.enter_context(tc.tile_pool(name="small", bufs=4))

            _borrow_ctr = [0]

            def st_borrow(name):
                # proj/v/outproj groups alternate between the dedicated fill
                # bank and the attn pool's spare slot so consecutive groups
                # don't serialize on one bank's evacuation
                _borrow_ctr[0] += 1
                if _borrow_ctr[0] % 2:
                    return fill_ps.tile([128, 512], F32, tag="fill", bufs=1,
                                        name=name)
                return at_ps.tile([128, 512], F32, tag="attn", bufs=3,
                                  name=name)

            # ---------------- work-group emitters ----------------------------
            def kq_group(dst2d, w_ab, b_sb, xs, hb, qci, name):
                qo, cw = QC[qci]
                wA, wB = w_ab
                ps = st_borrow(name)
                for c in range(DCH):
                    if hb == 0:
                        wslice = wA[:, c, :]
                    else:
                        wslice = wB[:, c, (hb - 1) * 128:hb * 128]
                    nc.tensor.matmul(
                        ps[:, :cw],
                        wslice,
                        xs[qci][c][:, 0:cw],
                        start=(c == 0),
                        stop=(not with_bias and c == DCH - 1),
                    )
                if with_bias:
                    nc.tensor.matmul(
                        ps[:, :cw],
                        b_sb[0:1, hb * 128:(hb + 1) * 128],
                        ones_r[0:1, :cw],
                        start=False, stop=True,
                    )
                nc.vector.tensor_copy(dst2d[:, qo:qo + cw], ps[:, :cw])

            def v_group(kt):
                ko, kh = KT[kt]
                qci = ko // 512
                lo = ko - qci * 512
                ps = st_borrow(f"vps{kt}")
                for c in range(DCH):
                    nc.tensor.matmul(
                        ps[:kh, :],
                        xkv_p[qci][c][:, lo:lo + kh],
                        wv_sb[c][:, :],
                        start=(c == 0),
                        stop=(not with_bias and c == DCH - 1),
                    )
                if with_bias:
                    nc.tensor.matmul(
                        ps[:kh, :],
                        ones_r[0:1, :kh],
                        bv_sb[0:1, :],
                        start=False, stop=True,
                    )
                nc.vector.tensor_copy(
                    v_t[kt][:kh, :, 0:64],
                    ps[:kh, :].rearrange("p (h c) -> p h c", c=64),
                )

            anorms = [[] for _ in range(NQC)]

            def op_group(ci, s, tail=False, alt=False):
                # both 512-col halves of the output row block, one DMA
                qo, cw = QC[ci]
                sw = min(128, cw - s * 128)
                osb = small.tile([128, 1024], F32, tag="os", bufs=3)
                if alt:
                    # tail: the S^T double-buffer is idle, borrow its banks
                    op3 = st_ps.tile([128, 2, 512], F32, tag="st", bufs=2,
                                     name=f"opa{ci}_{s}")
                    ops = [op3[:, 0, :], op3[:, 1, :]]
                else:
                    ops = None
                for dc in range(2):
                    op = ops[dc] if alt else st_borrow(f"op{ci}_{s}_{dc}")
                    for j in range(NPAIR):
                        nc.tensor.matmul(
                            op[:sw, :],
                            anorms[ci][j][:, s * 128:s * 128 + sw],
                            wo_sb[:, j, dc * 512:(dc + 1) * 512],
                            start=(j == 0), stop=(j == NPAIR - 1),
                        )
                    if tail and dc == 0:
                        nc.scalar.copy(osb[:sw, dc * 512:dc * 512 + 512],
                                       op[:sw, :])
                    else:
                        nc.vector.tensor_copy(
                            osb[:sw, dc * 512:dc * 512 + 512], op[:sw, :])
                nc.sync.dma_start(
                    out_d[qo + s * 128:qo + s * 128 + sw, :],
                    osb[:sw, :],
                )

            def normalize(ci, cw, pe_sb, po_sb, tail=False):
                an = an_pool.tile([128, 512], BF16, tag="an", bufs=8)
                rb_o = small.tile([64, 512], F32, tag="rb", bufs=4,
                                  name="rb_o")
                nc.vector.stream_shuffle(rb_o[0:32, :cw],
                                         po_sb[64:96, :cw], [0] * 32)
                nc.vector.stream_shuffle(rb_o[32:64, :cw],
                                         po_sb[64:96, :cw], [0] * 32)
                rr_o = small.tile([64, 512], F32, tag="rb", bufs=4,
                                  name="rr_o")
                nc.vector.reciprocal_approx_fast(rr_o[:, :cw],
                                                 rb_o[:, :cw])
                antmp = small.tile([64, 512], BF16, tag="antmp", bufs=2)
                nc.vector.tensor_mul(
                    antmp[:, :cw], po_sb[0:64, :cw], rr_o[:, :cw]
                )
                nc.sync.dma_start(an[64:128, :cw], antmp[:, :cw])
                rb_e = small.tile([64, 512], F32, tag="rb", bufs=4,
                                  name="rb_e")
                nc.vector.stream_shuffle(rb_e[0:32, :cw],
                                         pe_sb[64:96, :cw], [0] * 32)
                nc.vector.stream_shuffle(rb_e[32:64, :cw],
                                         pe_sb[64:96, :cw], [0] * 32)
                rr_e = small.tile([64, 512], F32, tag="rb", bufs=4,
                                  name="rr_e")
                nc.vector.reciprocal_approx_fast(rr_e[:, :cw], rb_e[:, :cw])
                nc.vector.tensor_mul(
                    an[0:64, :cw], pe_sb[0:64, :cw], rr_e[:, :cw]
                )
                anorms[ci].append(an)

            # ---------------- filler schedule --------------------------------
            # fillers[(ci, j, kt)] = list of thunks to emit at that step
            fillers = {}

            def pin(ci, j, kt, fn):
                fillers.setdefault((ci, j, kt), []).append(fn)

            def kg(hb, qci):
                return lambda: kq_group(kT[:, hb, :], (wkA, wkB), bk_sb, xkv_p,
                                        hb, qci, f"k{hb}_{qci}")

            def qg(hb, qci):
                return lambda: kq_group(qT_b[hb], (wqA, wqB), bq_sb, xq_p,
                                        hb, qci, f"q{hb}_{qci}")

            # v projections: group kt lands one step before attn@V consumes it
            for kt in range(NKT):
                pin(0, 0, kt, (lambda kt=kt: v_group(kt)))
            # kT block j cols: qc1 needed by S(kt=4), qc2 by S(kt=8) of pair j
            for j in range(1, NPAIR):
                pin(0, j, 1, kg(j, 1))
                pin(0, j, 5, kg(j, 2))
            pin(0, 0, 1, kg(0, 1))
            pin(0, 0, 5, kg(0, 2))
            # next pair's kT qc0 + qT qc0 late in the previous pair
            for j in range(1, NPAIR):
                pin(0, j - 1, 8, kg(j, 0))
                pin(0, j - 1, 10, qg(j, 0))
            # qT chunk 1 blocks: needed by chunk 1 pair j
            pin(0, 3, 2, qg(0, 1))
            pin(0, 3, 6, qg(1, 1))
            pin(1, 0, 2, qg(2, 1))
            pin(1, 1, 2, qg(3, 1))
            # qT chunk 2 blocks: needed by chunk 2 pair j
            pin(1, 2, 2, qg(0, 2))
            pin(1, 3, 2, qg(1, 2))
            pin(2, 0, 2, qg(2, 2))
            pin(2, 1, 2, qg(3, 2))
            # chunk 0 outproj into chunk 1; chunk 1 outproj into chunk 2
            for s in range(4):
                pin(1, s, 5, (lambda s=s: op_group(0, s)))
                pin(2, s, 5, (lambda s=s: op_group(1, s)))

            # ---------------- prefix: PE warmup + first k/q groups ------------
            warm = const.tile([128, 512], BF16, tag="warm")
            nc.vector.memset(warm[:], 0.0)
            wps = fill_ps.tile([128, 512], F32, tag="fill", bufs=1,
                               name="warm")
            NWARM = 28
            for i in range(NWARM):
                nc.tensor.matmul(wps[:], warm[:, 0:128], warm[:],
                                 start=(i == 0), stop=(i == NWARM - 1))
            kq_group(kT[:, 0, :], (wkA, wkB), bk_sb, xkv_p, 0, 0, "k0_0")
            kq_group(qT_b[0], (wqA, wqB), bq_sb, xq_p, 0, 0, "q0_0")

            # ---------------- attention ----------------
            for ci, (qo, cw) in enumerate(QC):
                for j in range(NPAIR):
                    pe_b = at_ps.tile([128, 512], F32, tag="attn", bufs=3,
                                      name=f"pe_{ci}_{j}")
                    po_b = at_ps.tile([128, 512], F32, tag="attn", bufs=3,
                                      name=f"po_{ci}_{j}")
                    pend = pend2 = pend3 = pend4 = None
                    for kt in range(NKT + 3):
                        if kt < NKT:
                            ko, kh = KT[kt]
                            st = st_ps.tile([128, 2, 512], F32, tag="st",
                                            bufs=2)
                            nc.tensor.matmul(
                                st[:kh, 0, :cw],
                                kT[0:64, j, ko:ko + kh],
                                qT_b[j][0:64, qo:qo + cw],
                                start=True, stop=True,
                            )
                            nc.tensor.matmul(
                                st[:kh, 1, :cw],
                                kT[64:128, j, ko:ko + kh],
                                qT_b[j][64:128, qo:qo + cw],
                                start=True, stop=True,
                            )
                            p = p_pool.tile([128, 2, 512], BF16, tag="p",
                                            bufs=8)
                            nc.scalar.activation(
                                p[:kh, :, :cw], st[:kh, :, :cw], AF.Exp,
                                scale=0.125,
                            )
                            pend = p
                        for fn in fillers.pop((ci, j, kt), ()):
                            fn()
                        if kt > 2:
                            kc = kt - 3
                            ko, kh = KT[kc]
                            nc.tensor.matmul(
                                pe_b[0:VW, :cw],
                                v_t[kc][0:kh, 2 * j, :],
                                pend4[0:kh, 0, :cw],
                                start=(kc == 0), stop=(kc == NKT - 1),
                            )
                            nc.tensor.matmul(
                                po_b[0:VW, :cw],
                                v_t[kc][0:kh, 2 * j + 1, :],
                                pend4[0:kh, 1, :cw],
                                start=(kc == 0), stop=(kc == NKT - 1),
                            )
                        pend4 = pend3
                        pend3 = pend2
                        pend2 = pend

                    # snapshot pe to SBUF on ScalarE so its psum bank
                    # releases before the next pair's po lands on it (at
                    # pool bufs=3 rotation); po is not reused until two
                    # pairs later so DVE reads it from psum directly.
                    # normalize = den broadcast via stream_shuffle, approx
                    # reciprocal, multiply; odd head first so the an[64:]
                    # recombine DMA issues as early as possible.
                    pe_sb = small.tile([96, 512], F32, tag="psb", bufs=2,
                                       name="pe_sb")
                    nc.scalar.copy(pe_sb[:, :cw], pe_b[0:96, :cw])
                    if (ci, j) == (NQC - 1, NPAIR - 1):
                        # last pair: also snapshot po (ScalarE is idle now)
                        # so every attn psum bank frees for the tail outproj
                        po_sb = small.tile([96, 512], F32, tag="psb", bufs=2,
                                           name="po_sb")
                        nc.scalar.copy(po_sb[:, :cw], po_b[0:96, :cw])
                        last_snap = (pe_sb, po_sb)
                    else:
                        normalize(ci, cw, pe_sb, po_b[0:96, :])

            # ---------------- tail: chunk 2 outproj ----------------
            # j=0..2 partials run on the PE while DVE normalizes the last
            # pair; only the j=3 contribution waits for its an tile
            qo2, cw2 = QC[2]
            op_banks = []
            for s in range(4):
                if s < 2:
                    t3 = st_ps.tile([128, 2, 512], F32, tag="st", bufs=2,
                                    name=f"opt{s}")
                    op_banks.append([t3[:, 0, :], t3[:, 1, :]])
                elif s == 2:
                    a0 = at_ps.tile([128, 512], F32, tag="attn", bufs=3,
                                    name="opt2a")
                    a1 = at_ps.tile([128, 512], F32, tag="attn", bufs=3,
                                    name="opt2b")
                    op_banks.append([a0[:, :], a1[:, :]])
                else:
                    a2 = at_ps.tile([128, 512], F32, tag="attn", bufs=3,
                                    name="opt3a")
                    f0 = fill_ps.tile([128, 512], F32, tag="fill", bufs=1,
                                      name="opt3b")
                    op_banks.append([a2[:, :], f0[:, :]])
            for s in range(4):
                sw = min(128, cw2 - s * 128)
                for dc in range(2):
                    for j in range(3):
                        nc.tensor.matmul(
                            op_banks[s][dc][:sw, :],
                            anorms[2][j][:, s * 128:s * 128 + sw],
                            wo_sb[:, j, dc * 512:(dc + 1) * 512],
                            start=(j == 0), stop=False,
                        )
            normalize(2, cw2, *last_snap, tail=True)
            for s in range(4):
                sw = min(128, cw2 - s * 128)
                osb = small.tile([128, 1024], F32, tag="os", bufs=3)
                for dc in range(2):
                    nc.tensor.matmul(
                        op_banks[s][dc][:sw, :],
                        anorms[2][3][:, s * 128:s * 128 + sw],
                        wo_sb[:, 3, dc * 512:(dc + 1) * 512],
                        start=False, stop=True,
                    )
                    if dc == 0:
                        nc.scalar.copy(osb[:sw, 0:512], op_banks[s][dc][:sw, :])
                    else:
                        nc.vector.tensor_copy(osb[:sw, 512:1024],
                                              op_banks[s][dc][:sw, :])
                nc.sync.dma_start(
                    out_d[qo2 + s * 128:qo2 + s * 128 + sw, :],
                    osb[:sw, :],
                )

    nc.compile()
    return nc


_NC = {}


def _get_nc(with_bias=False):
    if with_bias not in _NC:
        _NC[with_bias] = _build(with_bias)
    return _NC[with_bias]


def _shard_inputs(inputs_q, inputs_kv, Wq, bq, Wk, bk, Wv, bv, Wo, bo):
    ndt = ml_dtypes.bfloat16
    in_maps = []
    for b in range(B):
        xqT = np.ascontiguousarray(inputs_q[b].T).astype(ndt)
        xkvT = np.ascontiguousarray(inputs_kv[b].T).astype(ndt)
        for g in range(2):
            hs = slice(g * HG, (g + 1) * HG)
            in_maps.append({
                "xqT": xqT,
                "xkvT": xkvT,
                "wq": np.ascontiguousarray(Wq[:, hs, :].reshape(D, HHD)).astype(ndt),
                "wk": np.ascontiguousarray(Wk[:, hs, :].reshape(D, HHD)).astype(ndt),
                "wv": np.ascontiguousarray(Wv[:, hs, :].reshape(D, HHD)).astype(ndt),
                "wo": np.ascontiguousarray(Wo[hs].reshape(HHD, D)).astype(ndt),
                "bq": np.ascontiguousarray(bq[hs].reshape(1, HHD)).astype(ndt),
                "bk": np.ascontiguousarray(bk[hs].reshape(1, HHD)).astype(ndt),
                "bv": np.ascontiguousarray(bv[hs].reshape(1, HHD)).astype(ndt),
            })
    return in_maps


def _run(inputs, trace=False, trace_kwargs=None):
    inputs = {k: np.asarray(v) for k, v in inputs.items()}
    with_bias = bool(
        np.any(inputs["bq"]) or np.any(inputs["bk"]) or np.any(inputs["bv"])
    )
    nc = _get_nc(with_bias)
    in_maps = _shard_inputs(**inputs)
    res = run_bass_kernel_spmd(
        nc, in_maps, core_ids=list(range(2 * B)), trace=trace,
        **(trace_kwargs or {}),
    )
    bo = np.asarray(inputs["bo"], np.float32)
    out = np.empty((B, SEQ, D), np.float32)
    for b in range(B):
        out[b] = res.results[2 * b]["out"] + res.results[2 * b + 1]["out"] + bo
    return out, res


def kernel(**inputs):
    out, _ = _run(inputs, trace=False)
    return out
